# revision 88
# baseline (speedup 1.0000x reference)
"""Trainium2 Bass kernel: MultiHeadCrossAttentionWithBias.

Reference computation (per batch b):
  q_u = scale*(u_enc @ wq + wq_b); k/v from e_enc (and vice versa)
  ue_w = softmax(q_u k_e^T + bppw*bpp + bppb + mask*-inf); u_ctx = ue_w @ v_e
  u_update = u_ctx @ wo + wo_b                     (same mirrored for e)

Sharding: 8 fully independent attention units (batch b, direction d).
Core i = (d, b) handles one unit end-to-end; no collectives.

Design notes (~140us, vs the 154us fp16 baseline; PE column stream is
already optimal at 196608 moving cols ~= 107us, so every change here
attacks the head/tail/stall overheads around it):
 - bias+mask fused on host into one fp16 ebm_in = bppw*bpp + maskneg
   - 1.5 (uniform logit shift, cancels in softmax; keeps et inside
   fp16). Masked entries are -60000 -> exp == 0, which also implements
   the post-softmax re-mask. Per-(h,kc): et = exp(S) * exp(ebm_in).
 - whole attention value path in fp16 (er/ebm/et/v): 3x better relerr
   than the bf16 baseline and slightly faster DVE.
 - DMA: sync=q-side, scalar=k-side, gpsimd=v/wo; q-major projection
   emission matches that supply order. wqb|wkb|wob host-packed into one
   [128,12] tile, first on sync (it gates the first PSUM evictions).
 - output transposed [D, L] fp16 (host casts/transposes back): wob is
   then a per-partition ACT bias; bias-adds alternate ACT/DVE and
   stores alternate the two hw queues.
 - out-proj PSUM: tiles 0-5 on the released ps_s region (frees at the
   last exp), tiles 6-7 carved from the ps_c region and emitted last,
   so no out-proj matmul waits on the slow pair-3 eviction chain.
 - pair-3 1/den broadcast runs as 4 PE select-matmuls (sel rows x rcp
   row) into the ps_c region, overlapped with the out-proj p0-2 passes;
   pairs 0-2 keep the mid-stream DRAM-bounce broadcast (hidden).
 - evictions (den row + raw ctx copy, DVE) are deferred into the next
   head's kc==1 slot; the next head's first two QK/exp/mul groups are
   pre-emitted into the current head's PV-only tail slots. Mid-stream
   PE/ACT/DVE are all ~saturated at ~9us/head, a three-way tie, with
   the 3-deep score-PSUM ring bounding how far QK can run ahead.

Host prep is layout/precision only (transposes, slices, fp16 rounding,
mask/bias -> additive encoding); all FLOPs run on device.
"""

import numpy as np
from contextlib import ExitStack

import concourse.bass as bass
import concourse.tile as tile
import concourse.bacc as bacc
import concourse.mybir as mybir
from concourse import bass_utils

F32 = mybir.dt.float32
F16 = mybir.dt.float16
BF16 = mybir.dt.bfloat16
AF = mybir.ActivationFunctionType
ALU = mybir.AluOpType

B, L, D, H, HD = 4, 1024, 512, 8, 64
P = 128
FH = H * HD            # 512
SCALE = 1.0 / np.sqrt(HD)
N_CORES = 8
LAGS = (5, 4)          # kc lag between QK and PV streams (h==0, h>0)


def bcast_ap(dram_ap, parts):
    """Partition-step-0 broadcast AP over a DRAM row."""
    return bass.AP(tensor=dram_ap.tensor, offset=dram_ap.offset,
                   ap=[[0, parts]] + list(dram_ap.ap))


def build_module():
    nc = bacc.Bacc("TRN2", target_bir_lowering=False, debug=False)

    encQT_d = nc.dram_tensor("encQT", [D, L], F16, kind="ExternalInput")
    encKT_d = nc.dram_tensor("encKT", [D, L], F16, kind="ExternalInput")
    wq_d = nc.dram_tensor("wq", [D, FH], F16, kind="ExternalInput")
    wk_d = nc.dram_tensor("wk", [D, FH], F16, kind="ExternalInput")
    wv_d = nc.dram_tensor("wv", [D, FH], F16, kind="ExternalInput")
    wo_d = nc.dram_tensor("wo", [FH, D], F16, kind="ExternalInput")
    ebm_d = nc.dram_tensor("ebm", [L, L], F16, kind="ExternalInput")
    bcol_d = nc.dram_tensor("bcol", [P, 12], F32, kind="ExternalInput")
    wvb_d = nc.dram_tensor("wvb", [FH], F32, kind="ExternalInput")
    # transposed output [D, L]: wob varies along partitions, so the bias
    # add runs as an ACT per-partition bias (and host transposes back)
    out_d = nc.dram_tensor("out", [D, L], F16, kind="ExternalOutput")
    den_d = nc.dram_tensor("den_scratch", [H, L], F32, kind="Internal")

    with tile.TileContext(nc) as tc, ExitStack() as ctx:
        const = ctx.enter_context(tc.tile_pool(name="const", bufs=1))
        qkT_p = ctx.enter_context(tc.tile_pool(name="qkT", bufs=8))
        v_p = ctx.enter_context(tc.tile_pool(name="v", bufs=8))
        wo_p = ctx.enter_context(tc.tile_pool(name="wo", bufs=4))
        ebm_p = ctx.enter_context(tc.tile_pool(name="ebm", bufs=8))
        enc_p = ctx.enter_context(tc.tile_pool(name="enc", bufs=8))
        w_p = ctx.enter_context(tc.tile_pool(name="wqkv", bufs=12))
        cbt_p = ctx.enter_context(tc.tile_pool(name="cbtmp", bufs=8))
        ctxn_p = ctx.enter_context(tc.tile_pool(name="ctxn", bufs=4))
        den_p = ctx.enter_context(tc.tile_pool(name="den", bufs=4))
        er_p = ctx.enter_context(tc.tile_pool(name="er", bufs=6))
        e_p = ctx.enter_context(tc.tile_pool(name="e", bufs=9))
        rb_p = ctx.enter_context(tc.tile_pool(name="rb", bufs=4))
        ps_c = tc.alloc_tile_pool(name="ps_c", bufs=1, space="PSUM")
        ps_s = tc.alloc_tile_pool(name="ps_s", bufs=3, space="PSUM")

        # ---- small bias prep ----
        # host packs wqb|wkb|wob as one [128, 12] column tile: a single
        # contiguous DMA, first on the sync hw queue (it gates the first
        # ACT evictions; the gpsimd sw queue takes ~10us to start)
        bcol = const.tile([P, 12], F32)
        wqb_sc = const.tile([P, 4], F32)
        wkb_c = bcol[:, 4:8]
        wob_c = bcol[:, 8:12]
        wvb_bc = const.tile([P, FH], F32)
        nc.gpsimd.dma_start(wvb_bc[:], bcast_ap(wvb_d.ap(), P))
        # select rows for the pair-3 PE broadcast of 1/den (both on
        # partition 0): cols 0:128 keep partitions 0:64, cols 128:256
        # keep partitions 64:128
        sel_t = const.tile([1, 2 * P], F16)
        nc.vector.memset(sel_t[:], 0.0)
        nc.vector.memset(sel_t[0:1, 0:HD], 1.0)
        nc.vector.memset(sel_t[0:1, P + HD:2 * P], 1.0)

        # ---- input loads, first-use order, balanced across queues ----
        # sync (hw queue): q-side weights+enc, then even ebm tiles
        # scalar (hw queue): k-side weights+enc, then odd ebm tiles
        # gpsimd (sw queue): wv, wo
        eq, ek = [], []
        wq_t, wk_t, wv_t = [], [], []
        for dc in range(4):
            eng = nc.sync if dc % 2 == 0 else nc.scalar
            t = w_p.tile([P, FH], F16, tag="w", name=f"w_q{dc}")
            eng.dma_start(t[:], wq_d.ap()[dc * P:(dc + 1) * P, :])
            wq_t.append(t)
            te = enc_p.tile([P, L], F16, tag="enc", name=f"enc_q{dc}")
            eng.dma_start(te[:], encQT_d.ap()[dc * P:(dc + 1) * P, :])
            eq.append(te)
            if dc == 0:
                # the 128x48B bias-column DMA rides AFTER the first
                # matmul's operands: its many tiny descriptors would
                # otherwise delay the whole sync queue by ~2us, and its
                # consumer (first ACT eviction) isn't needed until ~15us
                nc.sync.dma_start(bcol[:], bcol_d.ap())
                nc.vector.tensor_scalar_mul(wqb_sc[:], bcol[:, 0:4],
                                            float(SCALE))
        for dc in range(4):
            eng = nc.sync if dc % 2 == 0 else nc.scalar
            t = w_p.tile([P, FH], F16, tag="w", name=f"w_k{dc}")
            eng.dma_start(t[:], wk_d.ap()[dc * P:(dc + 1) * P, :])
            wk_t.append(t)
            te = enc_p.tile([P, L], F16, tag="enc", name=f"enc_k{dc}")
            eng.dma_start(te[:], encKT_d.ap()[dc * P:(dc + 1) * P, :])
            ek.append(te)
        for dc in range(4):
            t = w_p.tile([P, FH], F16, tag="w", name=f"w_v{dc}")
            nc.gpsimd.dma_start(t[:], wv_d.ap()[dc * P:(dc + 1) * P, :])
            wv_t.append(t)
        ebm_in = {}
        for kc in range(8):
            b_t = cbt_p.tile([P, L], F16, tag="b", name=f"b{kc}")
            eng = nc.sync if kc % 2 == 0 else nc.scalar
            eng.dma_start(b_t[:], ebm_d.ap()[kc * P:(kc + 1) * P, :])
            ebm_in[kc] = b_t
        wo_t = []
        for p_ in range(4):
            t = wo_p.tile([P, D], F16, tag="wo", name=f"wo{p_}")
            nc.gpsimd.dma_start(t[:], wo_d.ap()[p_ * P:(p_ + 1) * P, :])
            wo_t.append(t)

        # ---- emission helpers ----
        qT, kT, v_aug = [None] * 4, [None] * 4, []
        ebm = [None] * 8

        def qk_pass(which, w_t, enc_t, out_list, bias, scl):
            # one full pass per side (q-major): matches the DMA supply
            # order, so the k-side stream has landed by the time the PE
            # reaches it
            for pc in range(4):
                o = qkT_p.tile([P, L], F16, tag="qkT", name=f"{which}T{pc}")
                for sh in range(2):
                    ps = ps_s.tile([P, 512], F32, tag="ps_s",
                                   name=f"ps_{which}{pc}_{sh}")
                    for dc in range(4):
                        nc.tensor.matmul(
                            ps[:],
                            w_t[dc][:, pc * P:(pc + 1) * P],
                            enc_t[dc][:, sh * 512:(sh + 1) * 512],
                            start=(dc == 0), stop=(dc == 3))
                    sl = slice(sh * 512, (sh + 1) * 512)
                    nc.scalar.activation(o[:, sl], ps[:], AF.Identity,
                                         bias=bias[:, pc:pc + 1], scale=scl)
                out_list[pc] = o

        def v_proj():
            for sc in range(8):
                ps = ps_s.tile([P, 512], F32, tag="ps_s", name=f"ps_v{sc}")
                for dc in range(4):
                    nc.tensor.matmul(ps[:], ek[dc][:, sc * P:(sc + 1) * P],
                                     wv_t[dc][:], start=(dc == 0),
                                     stop=(dc == 3))
                va = v_p.tile([P, H * (HD + 1)], F16, tag="v", name=f"v{sc}")
                vg = va[:].rearrange("p (h c) -> p h c", c=HD + 1)
                nc.vector.scalar_tensor_tensor(
                    vg[:, :, 0:HD],
                    ps[:].rearrange("p (h c) -> p h c", c=HD), 1.0,
                    wvb_bc[:].rearrange("p (h c) -> p h c", c=HD),
                    ALU.bypass, ALU.add)
                nc.vector.memset(vg[:, :, HD:HD + 1], 1.0)
                v_aug.append(va)

        def ebm_build(kc):
            e_t = ebm_p.tile([P, L], F16, tag="ebm", name=f"ebm{kc}")
            nc.scalar.activation(e_t[:], ebm_in[kc][:], AF.Exp)
            ebm[kc] = e_t

        def bounce(pc, rcp, rb):
            # 1/den partition-broadcast through DRAM, batched: ONE strided
            # DMA writes both den rows out, ONE 3-level-AP DMA broadcasts
            # them back across the two 64-partition halves of rb.
            for r_ in range(2):
                nc.sync.dma_start(
                    den_d.ap()[2 * pc + r_:2 * pc + r_ + 1, :],
                    rcp[32 * r_:32 * r_ + 1, :])
            nc.sync.dma_start(
                rb[0:HD, :], bcast_ap(den_d.ap()[2 * pc:2 * pc + 1, :], HD))
            nc.sync.dma_start(
                rb[HD:P, :], bcast_ap(den_d.ap()[2 * pc + 1:2 * pc + 2, :], HD))

        def evictions(h, c_ps):
            # evict raw ctx (DVE -> fp16, partition-shifted for odd heads)
            # and the den row; reciprocal + gpsimd partition_broadcast for
            # the partition-wise 1/den, then normalize ctxn in place.
            # Deferred into the next head's stream so these ops never
            # head-of-line block the exp/mult queues at head boundaries.
            o = (h % 2) * HD
            pc = h // 2
            if h % 2 == 0:
                den_sb = den_p.tile([33, L], F32, tag="den", name=f"den{pc}")
                nc.vector.memset(den_sb[:], 1.0)
                evictions.den_sb = den_sb
            den_sb = evictions.den_sb
            if h == H - 1:
                # last pair, all on DVE in dependency-optimal order: the
                # small den->recip->rcp16 chain FIRST (it gates the PE
                # select-matmul broadcast of 1/den in the out-proj), the
                # big ctx copy LAST (it only gates the ps_c region and
                # the normalize muls, needed later). ACT is still
                # draining the last exps and must not be on this path.
                nc.vector.tensor_copy(den_sb[32:33, :], c_ps[HD:HD + 1, :])
                rcp = den_p.tile([33, L], F32, tag="rcp", name=f"rcp{pc}")
                nc.vector.reciprocal_approx_fast(rcp[:], den_sb[:])
                rcp16 = den_p.tile([1, 2 * L], F16, tag="rcp", name="rcp16")
                nc.vector.tensor_copy(rcp16[0:1, 0:L], rcp[0:1, :])
                nc.vector.tensor_copy(rcp16[0:1, L:2 * L], rcp[32:33, :])
                nc.vector.tensor_copy(ctxn[pc][o:o + HD, :], c_ps[0:HD, :])
                evictions.rcp3 = rcp16
                return
            r0 = (h % 2) * 32
            nc.vector.tensor_copy(den_sb[r0:r0 + 1, :], c_ps[HD:HD + 1, :])
            nc.vector.tensor_copy(ctxn[pc][o:o + HD, :], c_ps[0:HD, :])
            if h % 2 == 1:
                rcp = den_p.tile([33, L], F32, tag="rcp", name=f"rcp{pc}")
                nc.vector.reciprocal_approx_fast(rcp[:], den_sb[:])
                rb = rb_p.tile([P, L], F32, tag="rb", name=f"rb{pc}")
                bounce(pc, rcp, rb)
                # normalize in place: ctxn *= 1/den
                nc.vector.tensor_mul(ctxn[pc][:], ctxn[pc][:], rb[:])

        def head(h):
            LAG = LAGS[0] if h == 0 else LAGS[1]
            o = (h % 2) * HD
            pc = h // 2
            # ctx accumulator: [65, 512] per qh, qh0 in free 0:512, qh1 in
            # 512:1024; den lands on partition 64 via the ones column.
            c_ps = ps_c.tile([P, L], F32, tag="ps_c", name=f"c_ps{h}")
            if h % 2 == 0:
                ctxn[pc] = ctxn_p.tile([P, L], F16, tag="ctxn",
                                       name=f"ctxn{pc}")
            def qk_emit(h2, kc2):
                pc2, o2 = h2 // 2, (h2 % 2) * HD
                s_ps = ps_s.tile([P, L], F32, tag="ps_s",
                                 name=f"s_ps_{h2}_{kc2}")
                for qh in range(2):
                    sl = slice(qh * 512, (qh + 1) * 512)
                    nc.tensor.matmul(
                        s_ps[:, sl],
                        kT[pc2][o2:o2 + HD, kc2 * P:(kc2 + 1) * P],
                        qT[pc2][o2:o2 + HD, sl],
                        start=True, stop=True)
                er = er_p.tile([P, L], F16, tag="er", name=f"er_{h2}_{kc2}")
                nc.scalar.activation(er[:], s_ps[:], AF.Exp)
                et = e_p.tile([P, L], F16, tag="e", name=f"e_{h2}_{kc2}")
                nc.vector.tensor_mul(et[:], er[:], ebm[kc2][:])
                return et

            e_ts = {}
            for kc in (0, 1):
                if (h, kc) in head.pre:
                    e_ts[kc] = head.pre.pop((h, kc))
            for kc in range(8 + LAG):
                if kc < 8 and kc not in e_ts:
                    e_ts[kc] = qk_emit(h, kc)
                if kc == 1 and head.pending is not None:
                    # previous head's eviction, emitted here so its DVE
                    # ops never head-of-line block this head's et stream
                    evictions(*head.pending)
                    head.pending = None
                if kc >= LAG:
                    kp = kc - LAG
                    for qh in range(2):
                        sl = slice(qh * 512, (qh + 1) * 512)
                        nc.tensor.matmul(
                            c_ps[0:HD + 1, sl],
                            v_aug[kp][:, h * (HD + 1):(h + 1) * (HD + 1)],
                            e_ts[kp][:, sl],
                            start=(kp == 0), stop=(kp == 7))
                # pre-emit the NEXT head's first two QK/exp/mul groups
                # into this head's PV-only tail slots: their exp/et are
                # ready long before the next head's first PV needs them
                if kc == 8 and h + 1 < H:
                    head.pre[(h + 1, 0)] = qk_emit(h + 1, 0)
                if kc == 10 and h + 1 < H:
                    head.pre[(h + 1, 1)] = qk_emit(h + 1, 1)
            head.pending = (h, c_ps)

        ctxn = [None] * 4
        head.pending = None
        head.pre = {}
        evictions.muls = []

        # ---- emission order ----
        qk_pass("q", wq_t, eq, qT, wqb_sc, float(SCALE))
        qk_pass("k", wk_t, ek, kT, wkb_c, 1.0)
        v_proj()
        for kc in range(8):
            ebm_build(kc)
        for h in range(H):
            head(h)
        evictions(*head.pending)
        head.pending = None

        # ---- output projection (transposed: out[e, q]) ----
        # p-major emission: all pair-0..2 matmuls first, so the PE only
        # waits on the last pair's normalize chain for the final 8
        # matmuls. Tile t = 4*qh + ec; bias-adds alternate ACT/DVE and
        # stores alternate the two hw DMA queues.
        # Tiles 0-5 land on the released ps_s region (free once the last
        # exp reads it). While their p0-2 passes run, the pair-3 1/den
        # broadcast runs as 4 PE select-matmuls into the ps_c region and
        # DVE normalizes ctxn[3]. Tiles 6-7 then reuse the ps_c region.
        ps_s.release()
        ps_o6 = tc.alloc_tile_pool(name="ps_o6", bufs=6, space="PSUM")
        with tc.tile_pool(name="outp", bufs=8) as out_p:
            o_ps = [ps_o6.tile([P, 512], F32, tag="ps_o", name=f"o_ps{t}")[:]
                    for t in range(6)]

            def mm(t, p_, stop=False):
                qh, ec = divmod(t, 4)
                nc.tensor.matmul(
                    o_ps[t],
                    wo_t[p_][:, ec * P:(ec + 1) * P],
                    ctxn[p_][:, qh * 512:(qh + 1) * 512],
                    start=(p_ == 0), stop=stop)

            for p_ in range(3):
                for t in range(6):
                    mm(t, p_)

            # pair-3 1/den partition-broadcast on the PE: rb[p, q] =
            # rcp[row(p), q] via two select-matmuls per q-half
            rcp3 = evictions.rcp3
            rb_ps = ps_c.tile([P, L], F32, tag="ps_c", name="rb_ps")
            for qh in range(2):
                sl = slice(qh * 512, (qh + 1) * 512)
                sl2 = slice(L + qh * 512, L + (qh + 1) * 512)
                nc.tensor.matmul(rb_ps[:, sl], sel_t[0:1, 0:P],
                                 rcp3[0:1, sl], start=True, stop=False)
                nc.tensor.matmul(rb_ps[:, sl], sel_t[0:1, P:2 * P],
                                 rcp3[0:1, sl2], start=False, stop=True)
            for qh in range(2):
                sl = slice(qh * 512, (qh + 1) * 512)
                nc.vector.tensor_mul(ctxn[3][:, sl], ctxn[3][:, sl],
                                     rb_ps[:, sl])

            o_ps67 = ps_c.tile([P, L], F32, tag="ps_c", name="o_ps67")
            o_ps += [o_ps67[:, 0:512], o_ps67[:, 512:1024]]

            # both q-halves of an ec land in one [128, 1024] tile: the
            # store packets are then full 2KB DRAM rows (half the packet
            # count of per-half stores), same matmul emission order
            ots = {}

            def finish(t):
                qh, ec = divmod(t, 4)
                mm(t, 3, stop=True)
                if qh == 0:
                    ots[ec] = out_p.tile([P, L], F16, tag="out",
                                         name=f"out{ec}")
                osl = ots[ec][:, qh * 512:(qh + 1) * 512]
                if t % 2 == 0:
                    nc.scalar.activation(osl, o_ps[t], AF.Identity,
                                         bias=wob_c[:, ec:ec + 1])
                else:
                    nc.vector.scalar_tensor_tensor(
                        osl, o_ps[t], wob_c[:, ec:ec + 1], wvb_bc[:],
                        ALU.add, ALU.bypass)
                if qh == 1:
                    qeng = nc.sync if ec % 2 == 0 else nc.scalar
                    qeng.dma_start(out_d.ap()[ec * P:(ec + 1) * P, :],
                                   ots[ec][:])

            for p_ in range(3):
                for t in range(6, 8):
                    mm(t, p_)
            for t in range(8):
                finish(t)
        ps_o6.release()
        ps_c.release()

    nc.compile()
    return nc


def shard_inputs(u_enc, e_enc, logit_bpp, ue_mask, eu_mask,
                 wq_k, wq_b, wk_k, wk_b, wv_k, wv_b, wo_k, wo_b,
                 bpp_w, bpp_b):
    """Build the 8 per-core input maps (layout + precision only).

    bpp_b is dropped: it shifts every logit in a row uniformly and
    cancels in softmax (as does the reference's +EPS).
    """
    u_enc = np.asarray(u_enc, np.float32)
    e_enc = np.asarray(e_enc, np.float32)
    bpp = np.asarray(logit_bpp, np.float32)
    ue_m = np.asarray(ue_mask).astype(np.float32)
    eu_m = np.asarray(eu_mask).astype(np.float32)
    bw = float(np.asarray(bpp_w, np.float32).reshape(()))

    def hf(x):
        return np.ascontiguousarray(x).astype(np.float16)

    com = dict(
        wq=hf(np.asarray(wq_k, np.float32).reshape(D, FH)),
        wk=hf(np.asarray(wk_k, np.float32).reshape(D, FH)),
        wv=hf(np.asarray(wv_k, np.float32).reshape(D, FH)),
        wo=hf(np.asarray(wo_k, np.float32).reshape(FH, D)),
        bcol=np.concatenate([
            np.asarray(wq_b, np.float32).reshape(4, P).T,
            np.asarray(wk_b, np.float32).reshape(4, P).T,
            np.asarray(wo_b, np.float32).reshape(4, P).T], axis=1).copy(),
        wvb=np.asarray(wv_b, np.float32).reshape(FH).copy(),
    )
    uT = [hf(u_enc[b].T) for b in range(B)]
    eT = [hf(e_enc[b].T) for b in range(B)]
    # -1.5 shifts all logits uniformly (cancels in softmax) and keeps
    # et = exp(s)*exp(cb) comfortably inside fp16 range
    bT = bw * bpp.T - 1.5
    bN = bw * bpp - 1.5
    # fused bias+mask, additive {bias, -60000} encoding, [k, q] orientation
    in_maps = []
    for i in range(N_CORES):
        d, b = divmod(i, B)
        if d == 0:      # u queries, e keys -> u_update[b]
            m = dict(encQT=uT[b], encKT=eT[b],
                     ebm=hf(bT + (ue_m[b, 0].T - 1.0) * 60000.0))
        else:           # e queries, u keys -> e_update[b]
            m = dict(encQT=eT[b], encKT=uT[b],
                     ebm=hf(bN + (eu_m[b, 0].T - 1.0) * 60000.0))
        m.update(com)
        in_maps.append(m)
    return in_maps


_NC = None


def kernel(**inputs):
    global _NC
    if _NC is None:
        _NC = build_module()
    in_maps = shard_inputs(**inputs)
    res = bass_utils.run_bass_kernel_spmd(
        _NC, in_maps, core_ids=list(range(N_CORES)))
    u_update = np.stack([res.results[b]["out"].T.astype(np.float32)
                         for b in range(B)])
    e_update = np.stack([res.results[B + b]["out"].T.astype(np.float32)
                         for b in range(B)])
    return u_update, e_update


if __name__ == "__main__":
    # single-core CoreSim check of one (direction, batch) unit
    from concourse.bass_interp import CoreSim

    rng = np.random.default_rng(0)
    u = rng.standard_normal((B, L, D)).astype(np.float32)
    e = rng.standard_normal((B, L, D)).astype(np.float32)
    bpp = rng.standard_normal((L, L)).astype(np.float32)
    uem = (rng.random((B, 1, L, L)) < 0.9)
    eum = (rng.random((B, 1, L, L)) < 0.9)
    w = 1.0 / np.sqrt(D)
    wq = (rng.standard_normal((D, H, HD)) * w).astype(np.float32)
    wk = (rng.standard_normal((D, H, HD)) * w).astype(np.float32)
    wv = (rng.standard_normal((D, H, HD)) * w).astype(np.float32)
    wo = (rng.standard_normal((H, HD, D)) / np.sqrt(FH)).astype(np.float32)
    zq = (rng.standard_normal((H, HD)) * 0.1).astype(np.float32)
    zo = (rng.standard_normal((D,)) * 0.1).astype(np.float32)

    nc = build_module()
    in_maps = shard_inputs(u, e, bpp, uem, eum, wq, zq, wk, zq, wv, zq,
                           wo, zo, np.float32(1.3), np.float32(-0.2))

    core = int(__import__("os").environ.get("CORE", "0"))
    sim = CoreSim(nc, trace=False)
    for k, vv in in_maps[core].items():
        sim.tensor(k)[:] = vv
    sim.simulate(check_with_hw=False)
    got = np.array(sim.tensor("out")).T.astype(np.float32)

    def ref_unit(encQ, encK, bias_qk, mask_qk):
        q = SCALE * (encQ @ wq.reshape(D, FH) + zq.reshape(FH))
        kk = encK @ wk.reshape(D, FH) + zq.reshape(FH)
        vv = encK @ wv.reshape(D, FH) + zq.reshape(FH)
        accum = np.zeros((L, D), np.float64)
        for h in range(H):
            qi = q[:, h * HD:(h + 1) * HD]
            ki = kk[:, h * HD:(h + 1) * HD]
            vi = vv[:, h * HD:(h + 1) * HD]
            s = qi @ ki.T + bias_qk
            s = np.where(mask_qk, s, -np.inf)
            s = s - s.max(-1, keepdims=True)
            p_ = np.exp(s)
            p_ /= p_.sum(-1, keepdims=True)
            accum += (p_ @ vi) @ wo[h]
        return (accum + zo).astype(np.float32)

    bq = 1.3 * bpp + -0.2
    if core < B:
        exp_out = ref_unit(u[core], e[core], bq, uem[core, 0])
    else:
        exp_out = ref_unit(e[core - B], u[core - B], bq.T, eum[core - B, 0])
    err = np.abs(got - exp_out).max() / np.abs(exp_out).max()
    print("unit relerr vs numpy:", err)
